# revision 19
# baseline (speedup 1.0000x reference)
"""Trainium2 Bass kernel for nn_Basic_Model_19078244729512.

Computes per-sample "returning rate" vectors p1, p2 from a [B, 25] grid
(reshaped [B, 5, 5]) of probabilities plus a mask tensor.

Sharding: pure data parallel over the batch. Each of the 8 cores gets
250112 rows (= 128 partitions x 1954); the global batch of 2,000,000 is
zero-padded by 896 rows so every core runs the same SPMD program.

Per-core layout: tiles of [128 partitions, F=240 rows/partition], inputs
DMA'd (HWDGE/sync ring, triple-buffered) as contiguous per-partition chunks
(row-major [f, c] with c = 0..24 the 5x5 grid). Intermediates live in
"k-major" F-blocks so every vector op processes all F samples of a
partition for several grid terms at once:

  prod tile (16 blocks):  T_k = p(4-k, k) * p(4-k, j) at block 4k + (j-1),
  valid j = k+1..4; invalid blocks zeroed (gpsimd memsets) so
  p1_j = sum_k P[4k + j-1] becomes two shifted vector adds.

  qs tile (9 blocks): [q40 q31 q22 q13 q04 | S1 S2 S3 S4] with
  q = 1-p (ACT engine), S_j = cumprod of q's; then
  p2_j = Q_j * (1 - S_j) * m_j via two fused scalar_tensor_tensor ops.

The device writes compact [N, 4] outputs (columns 1..4) via the gpsimd
(SWDGE) ring so output DMAs never stall the input ring; since all compute
is f32 and only the final store rounds, outputs are stored as fp16
(rel err ~3e-4, vs the 2e-2 gate) to halve output HBM traffic; the
constant-zero column 0 and the upcast back to fp32 are done host-side
during the gather. Compute is fully hidden behind the DMA stream (a
DMA-only ablation of the same traffic pattern measured the same), so
per-pass time is the 54 MB/core of HBM traffic at whatever rate the
container's HBM/axon tenancy sustains.

Input-side column slicing (only 15/25 pred and 4/25 mask columns are
used) was evaluated and REJECTED: per-row chunks would be 84 B (pred
[4:25)) and 52 B (mask [4:17)), and sub-512 B DMA descriptors take a
~2x read-modify-write penalty plus a ~7 ns/descriptor floor
(instruction_cost_v2.rs), capping sliced loads at ~120-190 GB/s --
slower than reading full 100 B rows contiguously at line rate.

Measured dead ends from the first session (kept behind build_nc flags):
mask loads on the second HWDGE ring (mask_dma_engine="scalar", ACT DMA
waits stall its Q5 compute), splitting each input DMA across both HWDGE
rings (in_split=True), merging output DMAs across tiles (out_group=2),
F=304/248/216. F=240 is SBUF-page-optimal: 240*25*4 B is exactly six
4 KiB pages per partition.

Second-session dead ends (same-process R8/R32 marginal A/B — axon
tenancy drifts +-25% between processes, so only same-window comparisons
are valid): merging p1+p2 into one [N,8] tensor (merged_out=True,
173.6us vs 162.7us control) and outputs on the scalar HWDGE ring
(out_dma_engine="scalar", 173.3us) both lose ~11us; the separate
fp16 outputs on the gpsimd SWDGE ring stand. fp16 vs fp32 outputs under
the same protocol: 155.6us vs 173.3us (-10%).
"""

import numpy as np

_B = 2_000_000
_NCORES = 8
_FTOT = 1954             # rows per partition per core
_NPC = 128 * _FTOT       # 250112 rows per core
_FMAX = 240              # tile rows per partition


def _legalize_waits(nc):
    """Split multi-wait sync_info into standalone EventSemaphore waits.

    The walrus build in this container encodes at most one sync-wait command
    per ISA instruction ("Too many sync wait commands" otherwise); hoist all
    but the last wait of each instruction into preceding single-wait
    EventSemaphore ops on the same engine (semantically identical: all waits
    are monotone semaphore conditions checked before issue).
    """
    import concourse.mybir as mybir

    for fn in nc.m.functions:
        for blk in fn.blocks:
            out = []
            for inst in blk.instructions:
                si = getattr(inst, "sync_info", None)
                waits = list(si.on_wait) if si is not None and si.on_wait else []
                if len(waits) > 1:
                    for k, w in enumerate(waits[:-1]):
                        out.append(mybir.InstEventSemaphore(
                            name=f"{inst.name}-w{k}",
                            engine=inst.engine,
                            ins=[], outs=[],
                            sync_info=mybir.SyncInfo(on_wait=[w], on_update=[]),
                        ))
                    inst.sync_info = mybir.SyncInfo(
                        on_wait=[waits[-1]],
                        on_update=list(si.on_update) if si.on_update else [],
                    )
                out.append(inst)
            blk.instructions = out
    return nc


def build_nc(ftot=_FTOT, fmax=_FMAX, bufs=2, legalize=True, reps=1,
             in_bufs=(4, 3), out_dma_engine="gpsimd", dma_only=False,
             tmp_bufs=1, mask_dma_engine="sync", out_group=1,
             in_split=False, memset_engine="gpsimd", out_dtype="f16",
             loop_reps=None, loop_unroll=2, merged_out=False):
    import concourse.bass as bass
    import concourse.mybir as mybir
    from concourse.tile import TileContext

    f32 = mybir.dt.float32
    if dma_only:
        out_dtype = "f32"   # ablation copies raw f32 tile bytes to the output
    fo = mybir.dt.float16 if out_dtype == "f16" else mybir.dt.float32
    MUL = mybir.AluOpType.mult
    ADD = mybir.AluOpType.add
    SUB = mybir.AluOpType.subtract
    COPY = mybir.ActivationFunctionType.Copy

    nrows = 128 * ftot
    nc = bass.Bass("TRN2", target_bir_lowering=False, debug=False)
    x = nc.dram_tensor("output", [nrows, 25], f32, kind="ExternalInput")
    mm = nc.dram_tensor("label_mask", [nrows, 25], f32, kind="ExternalInput")
    # device writes only columns 1..4 (as fp16 by default: all compute stays
    # fp32, only the final store rounds — rel err ~5e-4, and it halves output
    # HBM traffic); the constant-zero column 0 is assembled host-side in
    # kernel(), which also upcasts back to fp32
    if merged_out:
        assert not dma_only
        # single [N, 8] tensor: p1 cols 1..4 | p2 cols 1..4 — halves the
        # output DMA count and doubles each descriptor's contiguous span
        o12 = nc.dram_tensor("p12", [nrows, 8], fo, kind="ExternalOutput")
        o1 = o2 = None
    else:
        o1 = nc.dram_tensor("p1", [nrows, 4], fo, kind="ExternalOutput")
        o2 = nc.dram_tensor("p2", [nrows, 4], fo, kind="ExternalOutput")

    ib = in_bufs if isinstance(in_bufs, (list, tuple)) else [in_bufs, in_bufs]
    with TileContext(nc) as tc:
        with (
            tc.tile_pool(name="inp", bufs=ib[0] or bufs) as inp,
            tc.tile_pool(name="inp2", bufs=ib[1] or bufs) as inp2,
            tc.tile_pool(name="io", bufs=bufs) as io,
            tc.tile_pool(name="tmp", bufs=tmp_bufs or bufs) as tmp,
        ):
            engines = {"sync": nc.sync, "gpsimd": nc.gpsimd,
                       "scalar": nc.scalar}
            out_eng = engines[out_dma_engine]
            mask_eng = engines[mask_dma_engine]
            # reps>1: timing-only variant re-runs the whole pass
            chunks = []
            base = 0
            while base < ftot:
                chunks.append((base, min(fmax, ftot - base)))
                base += chunks[-1][1]
            # group equal-F chunks so their output DMAs merge into one
            groups, cur = [], []
            for bF in chunks:
                if cur and (len(cur) == out_group or cur[0][1] != bF[1]):
                    groups.append(cur)
                    cur = []
                cur.append(bF)
            if cur:
                groups.append(cur)
            def emit_grp(grp):
              Fg, Gn = grp[0][1], len(grp)
              if merged_out:
                  t12g = io.tile([128, 8 * Fg * Gn], fo, tag="t12")
              elif not dma_only:
                  t1g = io.tile([128, 4 * Fg * Gn], fo, tag="t1")
                  t2g = io.tile([128, 4 * Fg * Gn], fo, tag="t2")
              for gi, (base, F) in enumerate(grp):
                R0, RN = 128 * base, 128 * F
                x_sl = x[R0:R0 + RN, :].rearrange("(p f) c -> p (f c)", p=128)
                m_sl = mm[R0:R0 + RN, :].rearrange("(p f) c -> p (f c)", p=128)
                if dma_only:
                    o1_sl = o1[R0:R0 + RN, :].rearrange(
                        "(p f) c -> p (f c)", p=128)
                    o2_sl = o2[R0:R0 + RN, :].rearrange(
                        "(p f) c -> p (f c)", p=128)

                tin = inp.tile([128, F * 25], f32, tag="tin")
                tmk = inp2.tile([128, F * 25], f32, tag="tmk")
                if in_split:
                    # split each input transfer across both HWDGE rings
                    h = (F * 25) // 2
                    nc.sync.dma_start(tin[:, 0:h], x_sl[:, 0:h])
                    nc.scalar.dma_start(tin[:, h:], x_sl[:, h:])
                    nc.sync.dma_start(tmk[:, 0:h], m_sl[:, 0:h])
                    nc.scalar.dma_start(tmk[:, h:], m_sl[:, h:])
                else:
                    nc.sync.dma_start(tin[:], x_sl)
                    mask_eng.dma_start(tmk[:], m_sl)

                if dma_only:
                    out_eng.dma_start(o1_sl, tin[:, 0:F * 4])
                    out_eng.dma_start(o2_sl, tmk[:, 0:F * 4])
                    continue

                xin = tin[:].rearrange("p (f c) -> p f c", c=25)
                msk = tmk[:].rearrange("p (f c) -> p f c", c=25)
                mdiag = msk[:, :, 16:0:-4]          # m31 m22 m13 m04

                prod = tmp.tile([128, 16 * F], f32, tag="prod")
                pv = prod[:].rearrange("p (k f) -> p f k", f=F)
                qs = tmp.tile([128, 9 * F], f32, tag="qs")
                qv = qs[:].rearrange("p (k f) -> p f k", f=F)
                if merged_out:
                    o12v = t12g[:, gi * 8 * F:(gi + 1) * 8 * F].rearrange(
                        "p (f c) -> p f c", c=8)
                    o1v = o12v[:, :, 0:4]
                    o2v = o12v[:, :, 4:8]
                else:
                    o1v = t1g[:, gi * 4 * F:(gi + 1) * 4 * F].rearrange(
                        "p (f c) -> p f c", c=4)
                    o2v = t2g[:, gi * 4 * F:(gi + 1) * 4 * F].rearrange(
                        "p (f c) -> p f c", c=4)

                # ---- p1 ----
                # zero the padding blocks {4, 8, 9, 12, 13, 14}
                ms_eng = engines[memset_engine] if memset_engine != "vector" else nc.vector
                ms_eng.memset(pv[:, :, 4:5], 0.0)
                ms_eng.memset(pv[:, :, 8:10], 0.0)
                ms_eng.memset(pv[:, :, 12:15], 0.0)
                # products T_k: (broadcast anchor col) * (col range)
                for bc_c, c0, c1, blk in (
                    (20, 21, 25, 0),    # p40 * p41..p44  -> blocks 0..3
                    (16, 17, 20, 5),    # p31 * p32..p34  -> blocks 5..7
                    (12, 13, 15, 10),   # p22 * p23..p24  -> blocks 10..11
                    (8, 9, 10, 15),     # p13 * p14       -> block  15
                ):
                    n = c1 - c0
                    nc.vector.tensor_tensor(
                        pv[:, :, blk:blk + n],
                        xin[:, :, c0:c1],
                        xin[:, :, bc_c].broadcast_to((128, F, n)),
                        MUL,
                    )
                # mask the leading product of each chain: blocks {0,5,10,15}
                nc.vector.tensor_tensor(
                    pv[:, :, 0:16:5], pv[:, :, 0:16:5], mdiag, MUL
                )
                # p1_j = sum_k P[4k + j-1]: two shifted adds
                nc.vector.tensor_tensor(
                    pv[:, :, 0:8], pv[:, :, 0:8], pv[:, :, 8:16], ADD
                )
                nc.vector.tensor_tensor(
                    o1v[:, :, 0:4], pv[:, :, 0:4], pv[:, :, 4:8], ADD
                )

                # ---- p2 ----
                # q blocks 0..4 = 1 - [p40 p31 p22 p13 p04]  (ACT engine)
                nc.scalar.activation(
                    qv[:, :, 0:5], xin[:, :, 20:0:-4], COPY, bias=1.0, scale=-1.0
                )
                nc.scalar.activation(qv[:, :, 5:6], qv[:, :, 0:1], COPY)
                nc.vector.tensor_tensor(
                    qv[:, :, 6:7], qv[:, :, 5:6], qv[:, :, 1:2], MUL
                )
                nc.vector.tensor_tensor(
                    qv[:, :, 7:8], qv[:, :, 6:7], qv[:, :, 2:3], MUL
                )
                nc.vector.tensor_tensor(
                    qv[:, :, 8:9], qv[:, :, 7:8], qv[:, :, 3:4], MUL
                )
                # W_j = (S_j - 1) * Q_j   (in place over S blocks)
                nc.vector.scalar_tensor_tensor(
                    qv[:, :, 5:9], qv[:, :, 5:9], 1.0, qv[:, :, 1:5], SUB, MUL
                )
                # p2_j = (-W_j) * m_j
                nc.vector.scalar_tensor_tensor(
                    o2v[:, :, 0:4], qv[:, :, 5:9], -1.0, mdiag, MUL, MUL
                )

              if dma_only:
                  return
              R0g, RNg = 128 * grp[0][0], 128 * Fg * Gn
              if merged_out:
                  o12g_sl = o12[R0g:R0g + RNg, :].rearrange(
                      "(g p f) c -> p g (f c)", g=Gn, p=128)
                  out_eng.dma_start(o12g_sl, t12g[:])
              else:
                  o1g_sl = o1[R0g:R0g + RNg, :].rearrange(
                      "(g p f) c -> p g (f c)", g=Gn, p=128)
                  o2g_sl = o2[R0g:R0g + RNg, :].rearrange(
                      "(g p f) c -> p g (f c)", g=Gn, p=128)
                  out_eng.dma_start(o1g_sl, t1g[:])
                  out_eng.dma_start(o2g_sl, t2g[:])

            if loop_reps is None:
                for _ in range(reps):
                    for grp in groups:
                        emit_grp(grp)
            else:
                # timing-only: hardware loop over identical passes keeps the
                # program small (compile time/memory ~ O(loop_unroll), not
                # O(total passes)).  The For_i back-edge is a full barrier, so
                # measured per-pass time is a slight overestimate of the
                # free-running rate; loop_unroll passes per iteration amortize
                # it.
                with tc.For_i(0, loop_reps, 1):
                    for _ in range(loop_unroll):
                        for grp in groups:
                            emit_grp(grp)
    if loop_reps is not None:
        # For_i control flow leaves extended InstISA subclasses with empty
        # .instr; raw Bass skips the codegen pass Bacc.compile() would run
        mybir.codegen_inst_isa_subclasses(nc)
    return _legalize_waits(nc) if legalize else nc


def _run(output, label_mask, **spmd_kwargs):
    from concourse.bass_utils import run_bass_kernel_spmd

    output = np.ascontiguousarray(np.asarray(output), dtype=np.float32)
    label_mask = np.ascontiguousarray(np.asarray(label_mask), dtype=np.float32)
    assert output.shape == (_B, 25) and label_mask.shape == (_B, 25)

    pad = _NCORES * _NPC - _B
    xp = np.concatenate([output, np.zeros((pad, 25), np.float32)], axis=0)
    mp = np.concatenate([label_mask, np.zeros((pad, 25), np.float32)], axis=0)

    in_maps = [
        {
            "output": xp[i * _NPC:(i + 1) * _NPC],
            "label_mask": mp[i * _NPC:(i + 1) * _NPC],
        }
        for i in range(_NCORES)
    ]
    nc = build_nc()
    bres = run_bass_kernel_spmd(nc, in_maps, list(range(_NCORES)), **spmd_kwargs)
    res = bres.results
    p1 = np.zeros((_B, 5), np.float32)
    p2 = np.zeros((_B, 5), np.float32)
    # device outputs are fp16; assignment into the fp32 buffers upcasts
    if "p12" in res[0]:
        p12 = np.concatenate([np.asarray(r["p12"]) for r in res], axis=0)[:_B]
        p1[:, 1:5] = p12[:, 0:4]
        p2[:, 1:5] = p12[:, 4:8]
    else:
        p1[:, 1:5] = np.concatenate(
            [np.asarray(r["p1"]) for r in res], axis=0)[:_B]
        p2[:, 1:5] = np.concatenate(
            [np.asarray(r["p2"]) for r in res], axis=0)[:_B]
    return p1, p2, bres


def kernel(output, label_mask):
    p1, p2, _ = _run(output, label_mask)
    return p1, p2

